# revision 17
# baseline (speedup 1.0000x reference)
"""Trainium2 Bass kernel for an 8-batch transformer decoder block.

Sharding: data-parallel over batch N=8 -> one batch element per NeuronCore.
On-chip convention: activations are stored transposed (feature on the
partition dim, token on the free dim), so every matmul contracts over the
partition dim naturally and no on-device transposes are needed.  Attention
scores are built in [k, q] layout; softmax uses a constant shift (exact
for softmax) and the normalizer Z comes from a ones-column appended to V.
All heavy matmuls run fp16 with fp32 PSUM accumulation.
"""

import math
import time
from contextlib import ExitStack

import numpy as np

import concourse.bass as bass
import concourse.tile as tile
from concourse import bacc, mybir
from concourse import bass_utils

NB, SEQ, EMB, NH, FF = 8, 1024, 1024, 16, 4096
DH = EMB // NH  # 64
P = 128
QC = 512  # matmul moving free dim
NT = SEQ // P  # 8 tiles along a 1024 dim
NQC = SEQ // QC  # 2 q-chunks
FT = FF // P  # 32 f tiles
NEG = -30000.0  # additive mask value (fp16-safe, exp -> 0)
ESHIFT = -8.0  # constant softmax shift (cancels in normalization)
EPS = 1e-10

F16 = mybir.dt.float16
BF16 = mybir.dt.bfloat16
F32 = mybir.dt.float32

# bpack column layout (all per-partition bias/scale vectors, fp32)
_BQ1, _BK1, _BP1 = 0, 8, 16
_BQ2, _BK2, _BP2 = 24, 32, 40
_B1, _B2 = 48, 80
_G1, _S1, _G2, _S2, _G3, _S3 = 88, 96, 104, 112, 120, 128
NBCOL = 136


def _pbcast(ap, p):
    """Partition-broadcast read AP: replicate a [1, ...] AP across p partitions."""
    a = ap.copy()
    assert a.ap[0][1] == 1
    return bass.AP(tensor=a.tensor, offset=a.offset, ap=[[0, p]] + list(a.ap[1:]))


def _pbcast_pre(ap, p):
    """Prepend a broadcast partition dim to an AP."""
    a = ap.copy()
    return bass.AP(tensor=a.tensor, offset=a.offset, ap=[[0, p]] + list(a.ap))


def _emit(tc, ctx, io, self_blocks):
    nc = tc.nc

    const = ctx.enter_context(tc.tile_pool(name="const", bufs=1))
    acts = ctx.enter_context(tc.tile_pool(name="acts", bufs=1))
    wstr = ctx.enter_context(tc.tile_pool(name="wstr", bufs=8))
    exps = ctx.enter_context(tc.tile_pool(name="exps", bufs=12))
    tmps = ctx.enter_context(tc.tile_pool(name="tmps", bufs=3))
    stat = ctx.enter_context(tc.tile_pool(name="stat", bufs=4))
    bca = ctx.enter_context(tc.tile_pool(name="bca", bufs=2))
    dscr = ctx.enter_context(tc.tile_pool(name="dscr", bufs=6, space="DRAM"))
    psS = ctx.enter_context(tc.tile_pool(name="psS", bufs=4, space="PSUM"))
    psPr = ctx.enter_context(tc.tile_pool(name="psPr", bufs=3, space="PSUM"))
    psAV = psPr  # AV accumulators share the projection pool (disjoint phases)
    psSt = ctx.enter_context(tc.tile_pool(name="psSt", bufs=1, space="PSUM"))

    AF = mybir.ActivationFunctionType
    OP = mybir.AluOpType

    # ---- constants ----
    bpack = const.tile([P, NBCOL], F32, name="bpack", tag="bpack")
    nc.sync.dma_start(out=bpack, in_=io["bpack"])
    ones = const.tile([P, 1], F16, name="ones", tag="ones")
    nc.vector.memset(ones, 1.0)
    eshift = const.tile([P, 1], F32, name="eshift", tag="eshift")
    nc.vector.memset(eshift, ESHIFT)
    ones64 = const.tile([1, DH], BF16, name="ones64", tag="ones64")
    nc.vector.memset(ones64, 1.0)
    bvb = const.tile([P, 2, EMB], F16, name="bvb", tag="bvb")  # bv1/bv2 broadcast on partitions
    nc.sync.dma_start(out=bvb, in_=_pbcast_pre(io["bvrow"], P))

    def bcol(c):
        return bpack[:, c : c + 1]

    # ---------------- helpers ----------------
    def load_w(name, t, tag="w"):
        """Stream one [128, 1024] tile of a (1024,1024) fp16 weight."""
        wd = io[name].rearrange("(t p) n -> t p n", p=P)
        tl = wstr.tile([P, EMB], F16, tag=tag, bufs=8)
        nc.sync.dma_start(out=tl, in_=wd[t])
        return tl

    # ---- load transposed inputs, interleaved with the first proj's weights
    # so the first matmul doesn't wait behind the whole xT transfer ----
    xTd = io["xT"].rearrange("(t p) q -> t p q", p=P)
    xT = []
    wq1_pre = []
    for t in range(NT):
        wq1_pre.append(load_w("wq1", t))
        tl = acts.tile([P, SEQ], F16, name="xT", tag="xT", bufs=NT)
        nc.sync.dma_start(out=tl, in_=xTd[t])
        xT.append(tl)

    def proj_T(dst, src, wname, bc, w_pre=None):
        """dst[hd, q] = sum_m W[m, hd] * src[m, q] + b[hd]; dst: 8 result tiles."""
        w = w_pre if w_pre is not None else [load_w(wname, t) for t in range(NT)]
        for ot in range(NT):
            # both q-chunks accumulate together so consecutive matmuls share
            # the stationary operand (one weight load feeds two matmuls)
            pss = [psPr.tile([P, QC], F32, name="pr", tag="pr") for _ in range(NQC)]
            for mt in range(NT):
                for qc in range(NQC):
                    nc.tensor.matmul(
                        pss[qc],
                        lhsT=w[mt][:, ot * P : (ot + 1) * P],
                        rhs=src[mt][:, qc * QC : (qc + 1) * QC],
                        start=(mt == 0),
                        stop=(mt == NT - 1),
                    )
            for qc in range(NQC):
                nc.scalar.activation(
                    dst[ot][:, qc * QC : (qc + 1) * QC],
                    pss[qc],
                    AF.Identity,
                    bias=bcol(bc + ot),
                )

    def proj_V(vaug, src, wname, which):
        """vaug[kt][k, h, 0:64] = sum_m src[m, k]^T W[m, hd] + bv[hd] (free-dim bias)."""
        w = [load_w(wname, t) for t in range(NT)]
        for kt in range(NT):
            pss = [psPr.tile([P, QC], F32, name="pr", tag="pr") for _ in range(NQC)]
            for mt in range(NT):
                for hc in range(NQC):
                    nc.tensor.matmul(
                        pss[hc],
                        lhsT=src[mt][:, kt * P : (kt + 1) * P],
                        rhs=w[mt][:, hc * QC : (hc + 1) * QC],
                        start=(mt == 0),
                        stop=(mt == NT - 1),
                    )
            for hc in range(NQC):
                nc.vector.tensor_tensor(
                    out=vaug[kt][:, hc * 8 : (hc + 1) * 8, 0:64],
                    in0=pss[hc].rearrange("p (a b) -> p a b", a=8),
                    in1=bvb[:, which, hc * QC : (hc + 1) * QC].rearrange(
                        "p (a b) -> p a b", a=8
                    ),
                    op=OP.add,
                )

    def attention(QT, KT, vaug, YT, blocks, masked):
        """YT[hd, q] = softmax_k(KT_h^T QT_h / 8 [+mask]) -contracted- V.

        Heads are processed in even/odd pairs (row groups 0-63 / 64-127 of the
        PE array, so their score matmuls overlap); the AV accumulation of the
        previous (pair, qc) slot is interleaved block-by-block with the next
        slot's score matmuls so the exp latency never stalls the PE.
        """
        scl = 1.0 / math.sqrt(DH)
        mtiles = {}
        if masked:
            for qc in range(NQC):
                for kt, mixed in blocks[qc]:
                    if mixed and (kt, qc) not in mtiles:
                        mt = acts.tile([P, QC], F16, name="mtile", tag="mtile", bufs=8)
                        nc.sync.dma_start(
                            out=mt,
                            in_=io["mT"][
                                kt * P : (kt + 1) * P, qc * QC : (qc + 1) * QC
                            ],
                        )
                        mtiles[(kt, qc)] = mt

        def emit_scores(pt, qc, j):
            kt, mixed = blocks[qc][j]
            ps = psS.tile([P, 2, QC], F32, name="s", tag="s", bufs=2)
            for sub in (0, 1):
                base = sub * DH
                nc.tensor.matmul(
                    ps[:, sub, :],
                    lhsT=KT[pt][base : base + DH, kt * P : (kt + 1) * P],
                    rhs=QT[pt][base : base + DH, qc * QC : (qc + 1) * QC],
                    start=True,
                    stop=True,
                )
            e = exps.tile([P, 2, QC], F16, name="e", tag="e", bufs=12)
            # one exp over both heads' scores (2 banks, halves ACT op count);
            # exp(score/sqrt(dh) + shift); shift cancels in Z
            nc.scalar.activation(e, ps, AF.Exp, bias=eshift, scale=scl)
            if masked and mixed:
                m = mtiles[(kt, qc)]
                mb2 = bass.AP(
                    tensor=m.tensor,
                    offset=m.offset,
                    ap=[list(m.ap[0]), [0, 2], list(m.ap[1])],
                )
                nc.vector.tensor_tensor(out=e, in0=e, in1=mb2, op=OP.mult)
            return e

        def emit_av(state, j, nblk):
            (pt, qc, es, pys) = state
            kt, _ = blocks[qc][j]
            for sub in (0, 1):
                nc.tensor.matmul(
                    pys[sub],
                    lhsT=vaug[kt][:, 2 * pt + sub, 0 : DH + 1],
                    rhs=es[j][:, sub, :],
                    start=(j == 0),
                    stop=(j == nblk - 1),
                )

        def finish_a(state):
            """Evict AV psums (ACT) + 1/Z broadcast via DRAM bounce."""
            (pt, qc, es, pys) = state
            ab = []
            for sub in (0, 1):
                ytu = bca.tile([DH + 1, QC], F32, name="ytu", tag="ytu", bufs=4)
                nc.vector.tensor_scalar(
                    out=ytu, in0=pys[sub], scalar1=0.0, scalar2=None, op0=OP.add
                )
                # Z evicted separately to a partition-0 tile: the custom-DVE
                # approx reciprocal misreads its seed consts at partition
                # offsets > 0, so it must run at offset 0.
                z0 = stat.tile([1, QC], F32, name="z0", tag="z0", bufs=2)
                nc.vector.tensor_scalar(
                    out=z0, in0=pys[sub][DH : DH + 1, :], scalar1=0.0,
                    scalar2=None, op0=OP.add
                )
                zr = stat.tile([1, QC], F32, name="zr", tag="zr", bufs=2)
                nc.vector.reciprocal_approx_fast(zr, z0)
                zd = dscr.tile([1, QC], F32, name="zd", tag="zd")
                nc.sync.dma_start(out=zd, in_=zr)
                zb = bca.tile([DH, QC], F32, name="zb", tag="zb", bufs=4)
                nc.sync.dma_start(out=zb, in_=_pbcast(zd, DH))
                ab.append((ytu, zb))
            return (pt, qc, ab)

        def finish_b(fin):
            """Normalize YT (DVE); deferred one slot so the broadcast DMA has
            landed and the DVE FIFO never blocks on it."""
            (pt, qc, ab) = fin
            for sub in (0, 1):
                base = sub * DH
                ytu, zb = ab[sub]
                nc.gpsimd.tensor_tensor(
                    out=YT[pt][base : base + DH, qc * QC : (qc + 1) * QC],
                    in0=ytu[0:DH, :],
                    in1=zb,
                    op=OP.mult,
                )

        prev = None  # slot whose AV matmuls are pending
        pa = None  # slot evicted by finish_a, normalize pending
        for pt in range(NH // 2):
            for qc in range(NQC):
                blks = blocks[qc]
                es = []
                pys = [
                    psAV.tile([DH + 1, QC], F32, name="y", tag="pr") for _ in (0, 1)
                ]
                nprev = len(blocks[prev[1]]) if prev is not None else 0
                for j in range(max(len(blks), nprev)):
                    if j < len(blks):
                        es.append(emit_scores(pt, qc, j))
                    if prev is not None and j < nprev:
                        emit_av(prev, j, nprev)
                npa = finish_a(prev) if prev is not None else None
                if pa is not None:
                    finish_b(pa)
                pa = npa
                prev = (pt, qc, es, pys)
        for j in range(len(blocks[prev[1]])):
            emit_av(prev, j, len(blocks[prev[1]]))
        if pa is not None:
            finish_b(pa)
        finish_b(finish_a(prev))

    def layernorm(pre, out_tiles, gcol, scol, qc, out_dtype=F16, out_dram=None):
        """LN over the partition (feature) dim for one q-chunk.

        pre: list of 8 [P, SEQ] fp16 tiles (read slice qc).
        out_tiles: list of 8 dest tiles (write slice qc), or None with out_dram.
        """
        sl = slice(qc * QC, (qc + 1) * QC)
        pm = psSt.tile([1, QC], F32, name="st", tag="st")
        for mt in range(NT):
            nc.tensor.matmul(
                pm, lhsT=ones, rhs=pre[mt][:, sl], start=(mt == 0), stop=(mt == NT - 1)
            )
        mean = stat.tile([1, QC], F16, name="mean", tag="lnstat")
        nc.scalar.activation(mean, pm, AF.Identity, scale=1.0 / EMB)
        ps2 = psSt.tile([1, QC], F32, name="st", tag="st")
        for mt in range(NT):
            sq = tmps.tile([P, QC], F16, name="sq", tag="sq", bufs=2)
            nc.vector.tensor_mul(sq, pre[mt][:, sl], pre[mt][:, sl])
            nc.tensor.matmul(
                ps2, lhsT=ones, rhs=sq, start=(mt == 0), stop=(mt == NT - 1)
            )
        m2 = stat.tile([1, QC], F32, name="m2", tag="lnstat")
        nc.scalar.activation(m2, ps2, AF.Identity, scale=1.0 / EMB)
        var = stat.tile([1, QC], F32, name="var", tag="lnstat")
        nc.vector.tensor_tensor(out=var, in0=mean, in1=mean, op=OP.mult)
        nc.vector.tensor_tensor(out=var, in0=m2, in1=var, op=OP.subtract)
        # (mean is f16: its square's rounding is ~1e-7 absolute, negligible)
        # rstd = sqrt(1/var); the +EPS on std is 1e-10 relative, dropped
        rvar = stat.tile([1, QC], F32, name="rvar", tag="lnstat")
        nc.vector.reciprocal_approx_fast(rvar, var)
        rstd16 = stat.tile([1, QC], F16, name="rstd16", tag="lnstat")
        nc.scalar.activation(rstd16, rvar, AF.Sqrt)
        # broadcast mean/rstd across partitions via DRAM bounce (f16 for DVE 2x)
        md = dscr.tile([1, QC], F16, name="md", tag="md")
        nc.sync.dma_start(out=md, in_=mean)
        mb = bca.tile([P, QC], F16, name="mb", tag="mb", bufs=2)
        nc.sync.dma_start(out=mb, in_=_pbcast(md, P))
        rd = dscr.tile([1, QC], F16, name="rd", tag="rd")
        nc.sync.dma_start(out=rd, in_=rstd16)
        rb = bca.tile([P, QC], F16, name="rb", tag="rb", bufs=2)
        nc.sync.dma_start(out=rb, in_=_pbcast(rd, P))
        for mt in range(NT):
            eng = nc.gpsimd if (out_dram is not None and mt % 2) else nc.vector
            t1 = tmps.tile([P, QC], F16, name="lnt", tag="lnt", bufs=3)
            eng.tensor_tensor(out=t1, in0=pre[mt][:, sl], in1=mb, op=OP.subtract)
            eng.tensor_tensor(out=t1, in0=t1, in1=rb, op=OP.mult)
            if out_dram is None:
                nc.vector.tensor_scalar(
                    out=out_tiles[mt][:, sl],
                    in0=t1,
                    scalar1=bcol(gcol + mt),
                    scalar2=bcol(scol + mt),
                    op0=OP.mult,
                    op1=OP.add,
                )
            else:
                o = tmps.tile([P, QC], F16, name="otile", tag="otile", bufs=1)
                nc.vector.tensor_scalar(
                    out=o,
                    in0=t1,
                    scalar1=bcol(gcol + mt),
                    scalar2=bcol(scol + mt),
                    op0=OP.mult,
                    op1=OP.add,
                )
                nc.sync.dma_start(
                    out=out_dram[mt * P : (mt + 1) * P, qc * QC : (qc + 1) * QC], in_=o
                )

    def out_proj(YT, wname, bc, resid):
        """resid[mo, q] += sum_hd Wp[hd, mo] YT[hd, q] + bp[mo] (in place)."""
        pre = resid
        w = [load_w(wname, t) for t in range(NT)]
        for ot in range(NT):
            pss = [psPr.tile([P, QC], F32, name="pr", tag="pr") for _ in range(NQC)]
            for ht in range(NT):
                for qc in range(NQC):
                    nc.tensor.matmul(
                        pss[qc],
                        lhsT=w[ht][:, ot * P : (ot + 1) * P],
                        rhs=YT[ht][:, qc * QC : (qc + 1) * QC],
                        start=(ht == 0),
                        stop=(ht == NT - 1),
                    )
            for qc in range(NQC):
                ps = pss[qc]
                t = tmps.tile([P, QC], F16, name="lnt", tag="lnt", bufs=3)
                nc.vector.tensor_scalar(
                    out=t, in0=ps, scalar1=bcol(bc + ot), scalar2=None, op0=OP.add
                )
                nc.gpsimd.tensor_tensor(
                    out=pre[ot][:, qc * QC : (qc + 1) * QC],
                    in0=t,
                    in1=resid[ot][:, qc * QC : (qc + 1) * QC],
                    op=OP.add,
                )

    # ================= self-attention =================
    QT = [acts.tile([P, SEQ], F16, name="QT", tag="QT", bufs=NT) for _ in range(NT)]
    KT = [acts.tile([P, SEQ], F16, name="KT", tag="KT", bufs=NT) for _ in range(NT)]
    vaug = [acts.tile([P, NH, DH + 1], F16, name="vaug", tag="vaug", bufs=NT) for _ in range(NT)]
    for kt in range(NT):
        nc.vector.memset(vaug[kt][:, :, DH : DH + 1], 1.0)
    with nc.named_scope("selfQKV"):
        proj_T(QT, xT, "wq1", _BQ1, w_pre=wq1_pre)
        proj_T(KT, xT, "wk1", _BK1)
        proj_V(vaug, xT, "wv1", 0)
    YT = [acts.tile([P, SEQ], F16, name="YT", tag="YT", bufs=NT) for _ in range(NT)]
    with nc.named_scope("selfAttn"):
        attention(QT, KT, vaug, YT, self_blocks, masked=True)
    with nc.named_scope("selfOut"):
        out_proj(YT, "wp1", _BP1, xT)  # xT becomes o1pre in place
    o1T = [acts.tile([P, SEQ], F16, name="o1T", tag="o1T", bufs=NT) for _ in range(NT)]
    with nc.named_scope("ln1"):
        for qc in range(NQC):
            layernorm(xT, o1T, _G1, _S1, qc)

    # ================= cross-attention =================
    eT = []
    eTd = io["eT"].rearrange("(t p) q -> t p q", p=P)
    for t in range(NT):
        tl = acts.tile([P, SEQ], F16, name="eT", tag="eT", bufs=NT)
        nc.sync.dma_start(out=tl, in_=eTd[t])
        eT.append(tl)
    QT2 = [acts.tile([P, SEQ], F16, name="QT", tag="QT", bufs=NT) for _ in range(NT)]
    KT2 = [acts.tile([P, SEQ], F16, name="KT", tag="KT", bufs=NT) for _ in range(NT)]
    vaug2 = [acts.tile([P, NH, DH + 1], F16, name="vaug", tag="vaug", bufs=NT) for _ in range(NT)]
    for kt in range(NT):
        nc.vector.memset(vaug2[kt][:, :, DH : DH + 1], 1.0)
    # K/V first: they depend only on enc, so the PE works on them while the
    # DVE/ACT tail of LN1 finishes; Q (which needs o1T) comes last.
    with nc.named_scope("crossKV"):
        proj_T(KT2, eT, "wk2", _BK2)
        proj_V(vaug2, eT, "wv2", 1)
    with nc.named_scope("crossQ"):
        proj_T(QT2, o1T, "wq2", _BQ2)
    all_blocks = [[(kt, False) for kt in range(NT)] for _ in range(NQC)]
    YT2 = [acts.tile([P, SEQ], F16, name="YT", tag="YT", bufs=NT) for _ in range(NT)]
    with nc.named_scope("crossAttn"):
        attention(QT2, KT2, vaug2, YT2, all_blocks, masked=False)
    with nc.named_scope("crossOut"):
        out_proj(YT2, "wp2", _BP2, o1T)  # o1T becomes o2pre in place
    o2T = [acts.tile([P, SEQ], F16, name="o2T", tag="xT", bufs=NT) for _ in range(NT)]
    with nc.named_scope("ln2"):
        for qc in range(NQC):
            layernorm(o1T, o2T, _G2, _S2, qc)

    # ================= FFN =================
    FH = FT // 2  # 16 f-tiles per half
    o3pre = o2T  # o3pre overwrites o2T in place (after all reads of each chunk)
    ffn_scope = nc.named_scope("ffn")
    ffn_scope.__enter__()
    for qc in range(NQC):
        o3h = []  # fp32 partial sums for the first f-half
        for fh in range(2):
            # produce hT for this (qc, fh): 16 tiles of [P, QC] fp16
            hts = []
            for fi in range(FH):
                ft = fh * FH + fi
                w1 = wstr.tile([P, NT, P], F16, name="w", tag="w", bufs=8)
                nc.sync.dma_start(out=w1, in_=io["w1r"][ft])
                ph = psS.tile([P, QC], F32, name="s", tag="s", bufs=2)
                for mt in range(NT):
                    nc.tensor.matmul(
                        ph,
                        lhsT=w1[:, mt, :],
                        rhs=o2T[mt][:, qc * QC : (qc + 1) * QC],
                        start=(mt == 0),
                        stop=(mt == NT - 1),
                    )
                if fi % 2 == 0:
                    hpair = acts.tile([P, 2, QC], F16, name="QT", tag="QT", bufs=NT)
                h = hpair[:, fi % 2, :]
                nc.scalar.activation(h, ph, AF.Relu, bias=bcol(_B1 + ft))
                hts.append(h)
            # consume: o3 accumulation over this f-half
            for mo in range(NT):
                ps = psPr.tile([P, QC], F32, name="pr", tag="pr")
                w2c = []
                for half in range(2):
                    f0 = fh * FH + half * 8
                    w2 = wstr.tile([P, 8, P], F16, name="w2c", tag="w", bufs=8)
                    nc.sync.dma_start(
                        out=w2,
                        in_=io["w2b"][mo, f0 : f0 + 8].rearrange("f p c -> p f c"),
                    )
                    w2c.append(w2)
                for fi in range(FH):
                    nc.tensor.matmul(
                        ps,
                        lhsT=w2c[fi // 8][:, fi % 8, :],
                        rhs=hts[fi],
                        start=(fi == 0),
                        stop=(fi == FH - 1),
                    )
                if fh == 0:
                    o = acts.tile([P, QC], F32, name="eT", tag="eT", bufs=NT)
                    nc.vector.tensor_scalar(
                        out=o,
                        in0=ps,
                        scalar1=bcol(_B2 + mo),
                        scalar2=None,
                        op0=OP.add,
                    )
                    o3h.append(o)
                else:
                    t = tmps.tile([P, QC], F16, name="lnt", tag="lnt", bufs=3)
                    nc.vector.tensor_tensor(out=t, in0=ps, in1=o3h[mo], op=OP.add)
                    nc.vector.tensor_tensor(
                        out=o3pre[mo][:, qc * QC : (qc + 1) * QC],
                        in0=t,
                        in1=o2T[mo][:, qc * QC : (qc + 1) * QC],
                        op=OP.add,
                    )
        layernorm(o3pre, None, _G3, _S3, qc, out_dram=io["o3T"])
    ffn_scope.__exit__(None, None, None)


def _analyze_mask(mask):
    """Per q-chunk, the contributing k-tiles for self-attention and whether
    each needs the additive mask.  Must be consistent across all cores
    (falls back to fully-mixed otherwise)."""
    blocks = []
    any_mixed = False
    for qc in range(NQC):
        lst = []
        for kt in range(NT):
            sub = mask[:, qc * QC : (qc + 1) * QC, kt * P : (kt + 1) * P]
            if sub.all():
                continue  # fully masked on every core -> contributes nothing
            mixed = bool(sub.any())
            any_mixed = any_mixed or mixed
            lst.append((kt, mixed))
        blocks.append(lst)
    return blocks, any_mixed


def _build(self_blocks):
    nc = bacc.Bacc(
        "TRN2",
        target_bir_lowering=False,
        debug=False,
        num_devices=NB,
    )
    io = {}
    io["xT"] = nc.dram_tensor("xT", [EMB, SEQ], F16, kind="ExternalInput").ap()
    io["eT"] = nc.dram_tensor("eT", [EMB, SEQ], F16, kind="ExternalInput").ap()
    io["mT"] = nc.dram_tensor("mT", [SEQ, SEQ], F16, kind="ExternalInput").ap()
    for w in ("wq1", "wk1", "wv1", "wp1", "wq2", "wk2", "wv2", "wp2"):
        io[w] = nc.dram_tensor(w, [EMB, EMB], F16, kind="ExternalInput").ap()
    io["w1r"] = nc.dram_tensor("w1r", [FT, P, NT, P], F16, kind="ExternalInput").ap()
    io["w2b"] = nc.dram_tensor("w2b", [NT, FT, P, P], F16, kind="ExternalInput").ap()
    io["bpack"] = nc.dram_tensor("bpack", [P, NBCOL], F32, kind="ExternalInput").ap()
    io["bvrow"] = nc.dram_tensor("bvrow", [2, EMB], F16, kind="ExternalInput").ap()
    io["o3T"] = nc.dram_tensor("o3T", [EMB, SEQ], F16, kind="ExternalOutput").ap()

    with tile.TileContext(nc) as tc:
        with ExitStack() as ctx:
            _emit(tc, ctx, io, self_blocks)
    nc.compile()
    return nc


def _prep_inputs(inputs):
    """Host-side prep: per-core in_maps with transposed/retiled fp16 arrays."""
    f16 = np.float16
    dec, enc, mask = inputs["dec_inp"], inputs["enc_inp"], inputs["mask"]
    mask = np.asarray(mask).astype(bool)
    self_blocks, _ = _analyze_mask(mask)

    def headcat(w):  # [H, M, DH] -> [M, H*DH]
        return np.ascontiguousarray(
            np.asarray(w).transpose(1, 0, 2).reshape(EMB, EMB)
        ).astype(f16)

    shared = {}
    for tag in ("1", "2"):
        shared["wq" + tag] = headcat(inputs["Wq" + tag])
        shared["wk" + tag] = headcat(inputs["Wk" + tag])
        shared["wv" + tag] = headcat(inputs["Wv" + tag])
        shared["wp" + tag] = np.ascontiguousarray(inputs["Wp" + tag]).astype(f16)
    W1, W2 = np.asarray(inputs["W1"]), np.asarray(inputs["W2"])
    shared["w1r"] = np.ascontiguousarray(
        W1.reshape(NT, P, FT, P).transpose(2, 1, 0, 3)
    ).astype(f16)
    shared["w2b"] = np.ascontiguousarray(
        W2.reshape(FT, P, NT, P).transpose(2, 0, 1, 3)
    ).astype(f16)

    def cols(v):  # [n*128] -> [128, n] per-partition layout
        v = np.asarray(v).astype(np.float32)
        return v.reshape(-1, P).T

    bpack = np.zeros((P, NBCOL), np.float32)
    bpack[:, _BQ1:_BK1] = cols(np.asarray(inputs["bq1"]).reshape(-1))
    bpack[:, _BK1:_BP1] = cols(np.asarray(inputs["bk1"]).reshape(-1))
    bpack[:, _BP1:_BQ2] = cols(inputs["bp1"])
    bpack[:, _BQ2:_BK2] = cols(np.asarray(inputs["bq2"]).reshape(-1))
    bpack[:, _BK2:_BP2] = cols(np.asarray(inputs["bk2"]).reshape(-1))
    bpack[:, _BP2:_B1] = cols(inputs["bp2"])
    bpack[:, _B1:_B2] = cols(inputs["b1"])
    bpack[:, _B2 : _B2 + 8] = cols(inputs["b2"])
    bpack[:, _G1:_S1] = cols(inputs["g1"])
    bpack[:, _S1:_G2] = cols(inputs["s1"])
    bpack[:, _G2:_S2] = cols(inputs["g2"])
    bpack[:, _S2:_G3] = cols(inputs["s2"])
    bpack[:, _G3:_S3] = cols(inputs["g3"])
    bpack[:, _S3:NBCOL] = cols(inputs["s3"])
    shared["bpack"] = bpack
    shared["bvrow"] = np.stack(
        [
            np.asarray(inputs["bv1"]).reshape(-1),
            np.asarray(inputs["bv2"]).reshape(-1),
        ]
    ).astype(np.float16)

    in_maps = []
    for n in range(NB):
        m = dict(shared)
        m["xT"] = np.ascontiguousarray(np.asarray(dec[n]).T).astype(f16)
        m["eT"] = np.ascontiguousarray(np.asarray(enc[n]).T).astype(f16)
        m["mT"] = np.where(mask[n].T, np.float16(0.0), np.float16(1.0))
        in_maps.append(m)
    return in_maps, self_blocks


def kernel(**inputs):
    in_maps, self_blocks = _prep_inputs(inputs)
    nc = _build(self_blocks)
    # the first execution after another process crashed the device can hit a
    # transient NRT_EXEC_UNIT_UNRECOVERABLE; a retry has always recovered it
    last = None
    for attempt in range(3):
        try:
            res = bass_utils.run_bass_kernel_spmd(
                nc, in_maps, core_ids=list(range(NB))
            )
            break
        except Exception as e:  # noqa: BLE001
            last = e
            time.sleep(3)
    else:
        raise last
    out = np.stack([np.asarray(r["o3T"]).T for r in res.results])
    return out.astype(np.float32)



# revision 18
# speedup vs baseline: 1.0189x; 1.0189x over previous
"""Trainium2 Bass kernel for an 8-batch transformer decoder block.

Sharding: data-parallel over batch N=8 -> one batch element per NeuronCore.
On-chip convention: activations are stored transposed (feature on the
partition dim, token on the free dim), so every matmul contracts over the
partition dim naturally and no on-device transposes are needed.  Attention
scores are built in [k, q] layout; softmax uses a constant shift (exact
for softmax) and the normalizer Z comes from a ones-column appended to V.
All heavy matmuls run fp16 with fp32 PSUM accumulation.
"""

import math
import time
from contextlib import ExitStack

import numpy as np

import concourse.bass as bass
import concourse.tile as tile
from concourse import bacc, mybir
from concourse import bass_utils

NB, SEQ, EMB, NH, FF = 8, 1024, 1024, 16, 4096
DH = EMB // NH  # 64
P = 128
QC = 512  # matmul moving free dim
NT = SEQ // P  # 8 tiles along a 1024 dim
NQC = SEQ // QC  # 2 q-chunks
FT = FF // P  # 32 f tiles
NEG = -30000.0  # additive mask value (fp16-safe, exp -> 0)
ESHIFT = -8.0  # constant softmax shift (cancels in normalization)
EPS = 1e-10

F16 = mybir.dt.float16
BF16 = mybir.dt.bfloat16
F32 = mybir.dt.float32

# bpack column layout (all per-partition bias/scale vectors, fp32)
_BQ1, _BK1, _BP1 = 0, 8, 16
_BQ2, _BK2, _BP2 = 24, 32, 40
_B1, _B2 = 48, 80
_G1, _S1, _G2, _S2, _G3, _S3 = 88, 96, 104, 112, 120, 128
NBCOL = 136


def _pbcast(ap, p):
    """Partition-broadcast read AP: replicate a [1, ...] AP across p partitions."""
    a = ap.copy()
    assert a.ap[0][1] == 1
    return bass.AP(tensor=a.tensor, offset=a.offset, ap=[[0, p]] + list(a.ap[1:]))


def _pbcast_pre(ap, p):
    """Prepend a broadcast partition dim to an AP."""
    a = ap.copy()
    return bass.AP(tensor=a.tensor, offset=a.offset, ap=[[0, p]] + list(a.ap))


def _emit(tc, ctx, io, self_blocks):
    nc = tc.nc

    const = ctx.enter_context(tc.tile_pool(name="const", bufs=1))
    acts = ctx.enter_context(tc.tile_pool(name="acts", bufs=1))
    wstr = ctx.enter_context(tc.tile_pool(name="wstr", bufs=8))
    exps = ctx.enter_context(tc.tile_pool(name="exps", bufs=11))
    tmps = ctx.enter_context(tc.tile_pool(name="tmps", bufs=3))
    stat = ctx.enter_context(tc.tile_pool(name="stat", bufs=4))
    bca = ctx.enter_context(tc.tile_pool(name="bca", bufs=2))
    dscr = ctx.enter_context(tc.tile_pool(name="dscr", bufs=6, space="DRAM"))
    psS = ctx.enter_context(tc.tile_pool(name="psS", bufs=4, space="PSUM"))
    psPr = ctx.enter_context(tc.tile_pool(name="psPr", bufs=3, space="PSUM"))
    psAV = psPr  # AV accumulators share the projection pool (disjoint phases)
    psSt = ctx.enter_context(tc.tile_pool(name="psSt", bufs=1, space="PSUM"))

    AF = mybir.ActivationFunctionType
    OP = mybir.AluOpType

    # ---- constants ----
    bpack = const.tile([P, NBCOL], F32, name="bpack", tag="bpack")
    nc.sync.dma_start(out=bpack, in_=io["bpack"])
    ones = const.tile([P, 1], F16, name="ones", tag="ones")
    nc.vector.memset(ones, 1.0)
    eshift = const.tile([P, 1], F32, name="eshift", tag="eshift")
    nc.vector.memset(eshift, ESHIFT)
    ones64 = const.tile([1, DH], BF16, name="ones64", tag="ones64")
    nc.vector.memset(ones64, 1.0)
    bvb = const.tile([P, 2, EMB], F16, name="bvb", tag="bvb")  # bv1/bv2 broadcast on partitions
    nc.sync.dma_start(out=bvb, in_=_pbcast_pre(io["bvrow"], P))

    def bcol(c):
        return bpack[:, c : c + 1]

    # ---------------- helpers ----------------
    def load_w(name, t, tag="w"):
        """Stream one [128, 1024] tile of a (1024,1024) fp16 weight."""
        wd = io[name].rearrange("(t p) n -> t p n", p=P)
        tl = wstr.tile([P, EMB], F16, tag=tag, bufs=8)
        nc.sync.dma_start(out=tl, in_=wd[t])
        return tl

    # ---- load transposed inputs, interleaved with the first proj's weights
    # so the first matmul doesn't wait behind the whole xT transfer ----
    xTd = io["xT"].rearrange("(t p) q -> t p q", p=P)
    xT = []
    wq1_pre = []
    for t in range(NT):
        wq1_pre.append(load_w("wq1", t))
        tl = acts.tile([P, SEQ], F16, name="xT", tag="xT", bufs=NT)
        nc.sync.dma_start(out=tl, in_=xTd[t])
        xT.append(tl)

    def proj_T(dst, src, wname, bc, w_pre=None):
        """dst[hd, q] = sum_m W[m, hd] * src[m, q] + b[hd]; dst: 8 result tiles."""
        w = w_pre if w_pre is not None else [load_w(wname, t) for t in range(NT)]
        for ot in range(NT):
            # both q-chunks accumulate together so consecutive matmuls share
            # the stationary operand (one weight load feeds two matmuls)
            pss = [psPr.tile([P, QC], F32, name="pr", tag="pr") for _ in range(NQC)]
            for mt in range(NT):
                for qc in range(NQC):
                    nc.tensor.matmul(
                        pss[qc],
                        lhsT=w[mt][:, ot * P : (ot + 1) * P],
                        rhs=src[mt][:, qc * QC : (qc + 1) * QC],
                        start=(mt == 0),
                        stop=(mt == NT - 1),
                    )
            for qc in range(NQC):
                nc.scalar.activation(
                    dst[ot][:, qc * QC : (qc + 1) * QC],
                    pss[qc],
                    AF.Identity,
                    bias=bcol(bc + ot),
                )

    def proj_V(vaug, src, wname, which):
        """vaug[kt][k, h, 0:64] = sum_m src[m, k]^T W[m, hd] + bv[hd] (free-dim bias)."""
        w = [load_w(wname, t) for t in range(NT)]
        for kt in range(NT):
            pss = [psPr.tile([P, QC], F32, name="pr", tag="pr") for _ in range(NQC)]
            for mt in range(NT):
                for hc in range(NQC):
                    nc.tensor.matmul(
                        pss[hc],
                        lhsT=src[mt][:, kt * P : (kt + 1) * P],
                        rhs=w[mt][:, hc * QC : (hc + 1) * QC],
                        start=(mt == 0),
                        stop=(mt == NT - 1),
                    )
            for hc in range(NQC):
                nc.vector.tensor_tensor(
                    out=vaug[kt][:, hc * 8 : (hc + 1) * 8, 0:64],
                    in0=pss[hc].rearrange("p (a b) -> p a b", a=8),
                    in1=bvb[:, which, hc * QC : (hc + 1) * QC].rearrange(
                        "p (a b) -> p a b", a=8
                    ),
                    op=OP.add,
                )

    def attention(QT, KT, vaug, YT, blocks, masked):
        """YT[hd, q] = softmax_k(KT_h^T QT_h / 8 [+mask]) -contracted- V.

        Heads are processed in even/odd pairs (row groups 0-63 / 64-127 of the
        PE array, so their score matmuls overlap); the AV accumulation of the
        previous (pair, qc) slot is interleaved block-by-block with the next
        slot's score matmuls so the exp latency never stalls the PE.
        """
        scl = 1.0 / math.sqrt(DH)
        mtiles = {}
        if masked:
            for qc in range(NQC):
                for kt, mixed in blocks[qc]:
                    if mixed and (kt, qc) not in mtiles:
                        mt = acts.tile([P, QC], F16, name="mtile", tag="mtile", bufs=8)
                        nc.sync.dma_start(
                            out=mt,
                            in_=io["mT"][
                                kt * P : (kt + 1) * P, qc * QC : (qc + 1) * QC
                            ],
                        )
                        mtiles[(kt, qc)] = mt

        def emit_scores(pt, qc, j):
            kt, mixed = blocks[qc][j]
            ps = psS.tile([P, 2, QC], F32, name="s", tag="s", bufs=2)
            for sub in (0, 1):
                base = sub * DH
                nc.tensor.matmul(
                    ps[:, sub, :],
                    lhsT=KT[pt][base : base + DH, kt * P : (kt + 1) * P],
                    rhs=QT[pt][base : base + DH, qc * QC : (qc + 1) * QC],
                    start=True,
                    stop=True,
                )
            e = exps.tile([P, 2, QC], F16, name="e", tag="e", bufs=11)
            # one exp over both heads' scores (2 banks, halves ACT op count);
            # exp(score/sqrt(dh) + shift); shift cancels in Z
            nc.scalar.activation(e, ps, AF.Exp, bias=eshift, scale=scl)
            if masked and mixed:
                m = mtiles[(kt, qc)]
                mb2 = bass.AP(
                    tensor=m.tensor,
                    offset=m.offset,
                    ap=[list(m.ap[0]), [0, 2], list(m.ap[1])],
                )
                nc.vector.tensor_tensor(out=e, in0=e, in1=mb2, op=OP.mult)
            return e

        def emit_av(state, j, nblk):
            (pt, qc, es, pys) = state
            kt, _ = blocks[qc][j]
            for sub in (0, 1):
                nc.tensor.matmul(
                    pys[sub],
                    lhsT=vaug[kt][:, 2 * pt + sub, 0 : DH + 1],
                    rhs=es[j][:, sub, :],
                    start=(j == 0),
                    stop=(j == nblk - 1),
                )

        def finish_a(state):
            """Evict AV psums (ACT) + 1/Z broadcast via DRAM bounce."""
            (pt, qc, es, pys) = state
            ab = []
            for sub in (0, 1):
                ytu = bca.tile([DH + 1, QC], F32, name="ytu", tag="ytu", bufs=4)
                nc.vector.tensor_scalar(
                    out=ytu, in0=pys[sub], scalar1=0.0, scalar2=None, op0=OP.add
                )
                # Z evicted separately to a partition-0 tile: the custom-DVE
                # approx reciprocal misreads its seed consts at partition
                # offsets > 0, so it must run at offset 0.
                z0 = stat.tile([1, QC], F32, name="z0", tag="z0", bufs=2)
                nc.vector.tensor_scalar(
                    out=z0, in0=pys[sub][DH : DH + 1, :], scalar1=0.0,
                    scalar2=None, op0=OP.add
                )
                zr = stat.tile([1, QC], F32, name="zr", tag="zr", bufs=2)
                nc.vector.reciprocal_approx_fast(zr, z0)
                zd = dscr.tile([1, QC], F32, name="zd", tag="zd")
                nc.sync.dma_start(out=zd, in_=zr)
                zb = bca.tile([DH, QC], F32, name="zb", tag="zb", bufs=4)
                nc.sync.dma_start(out=zb, in_=_pbcast(zd, DH))
                ab.append((ytu, zb))
            return (pt, qc, ab)

        def finish_b(fin):
            """Normalize YT (DVE); deferred one slot so the broadcast DMA has
            landed and the DVE FIFO never blocks on it."""
            (pt, qc, ab) = fin
            for sub in (0, 1):
                base = sub * DH
                ytu, zb = ab[sub]
                nc.gpsimd.tensor_tensor(
                    out=YT[pt][base : base + DH, qc * QC : (qc + 1) * QC],
                    in0=ytu[0:DH, :],
                    in1=zb,
                    op=OP.mult,
                )

        prev = None  # slot whose AV matmuls are pending
        pa = None  # slot evicted by finish_a, normalize pending
        for pt in range(NH // 2):
            for qc in range(NQC):
                blks = blocks[qc]
                es = []
                pys = [
                    psAV.tile([DH + 1, QC], F32, name="y", tag="pr") for _ in (0, 1)
                ]
                nprev = len(blocks[prev[1]]) if prev is not None else 0
                for j in range(max(len(blks), nprev)):
                    if j < len(blks):
                        es.append(emit_scores(pt, qc, j))
                    if prev is not None and j < nprev:
                        emit_av(prev, j, nprev)
                npa = finish_a(prev) if prev is not None else None
                if pa is not None:
                    finish_b(pa)
                pa = npa
                prev = (pt, qc, es, pys)
        for j in range(len(blocks[prev[1]])):
            emit_av(prev, j, len(blocks[prev[1]]))
        if pa is not None:
            finish_b(pa)
        finish_b(finish_a(prev))

    def layernorm(pre, out_tiles, gcol, scol, qc, out_dtype=F16, out_dram=None):
        """LN over the partition (feature) dim for one q-chunk.

        pre: list of 8 [P, SEQ] fp16 tiles (read slice qc).
        out_tiles: list of 8 dest tiles (write slice qc), or None with out_dram.
        """
        sl = slice(qc * QC, (qc + 1) * QC)
        pm = psSt.tile([1, QC], F32, name="st", tag="st")
        for mt in range(NT):
            nc.tensor.matmul(
                pm, lhsT=ones, rhs=pre[mt][:, sl], start=(mt == 0), stop=(mt == NT - 1)
            )
        mean = stat.tile([1, QC], F16, name="mean", tag="lnstat")
        nc.scalar.activation(mean, pm, AF.Identity, scale=1.0 / EMB)
        ps2 = psSt.tile([1, QC], F32, name="st", tag="st")
        for mt in range(NT):
            sq = tmps.tile([P, QC], F16, name="sq", tag="sq", bufs=2)
            nc.vector.tensor_mul(sq, pre[mt][:, sl], pre[mt][:, sl])
            nc.tensor.matmul(
                ps2, lhsT=ones, rhs=sq, start=(mt == 0), stop=(mt == NT - 1)
            )
        m2 = stat.tile([1, QC], F32, name="m2", tag="lnstat")
        nc.scalar.activation(m2, ps2, AF.Identity, scale=1.0 / EMB)
        var = stat.tile([1, QC], F32, name="var", tag="lnstat")
        nc.vector.tensor_tensor(out=var, in0=mean, in1=mean, op=OP.mult)
        nc.vector.tensor_tensor(out=var, in0=m2, in1=var, op=OP.subtract)
        # (mean is f16: its square's rounding is ~1e-7 absolute, negligible)
        # rstd = sqrt(1/var); the +EPS on std is 1e-10 relative, dropped
        rvar = stat.tile([1, QC], F32, name="rvar", tag="lnstat")
        nc.vector.reciprocal_approx_fast(rvar, var)
        rstd16 = stat.tile([1, QC], F16, name="rstd16", tag="lnstat")
        nc.scalar.activation(rstd16, rvar, AF.Sqrt)
        # broadcast mean/rstd across partitions via DRAM bounce (f16 for DVE 2x)
        md = dscr.tile([1, QC], F16, name="md", tag="md")
        nc.sync.dma_start(out=md, in_=mean)
        mb = bca.tile([P, QC], F16, name="mb", tag="mb", bufs=2)
        nc.sync.dma_start(out=mb, in_=_pbcast(md, P))
        rd = dscr.tile([1, QC], F16, name="rd", tag="rd")
        nc.sync.dma_start(out=rd, in_=rstd16)
        rb = bca.tile([P, QC], F16, name="rb", tag="rb", bufs=2)
        nc.sync.dma_start(out=rb, in_=_pbcast(rd, P))
        for mt in range(NT):
            eng = nc.gpsimd if (out_dram is not None and mt % 2) else nc.vector
            t1 = tmps.tile([P, QC], F16, name="lnt", tag="lnt", bufs=3)
            eng.tensor_tensor(out=t1, in0=pre[mt][:, sl], in1=mb, op=OP.subtract)
            eng.tensor_tensor(out=t1, in0=t1, in1=rb, op=OP.mult)
            if out_dram is None:
                nc.vector.tensor_scalar(
                    out=out_tiles[mt][:, sl],
                    in0=t1,
                    scalar1=bcol(gcol + mt),
                    scalar2=bcol(scol + mt),
                    op0=OP.mult,
                    op1=OP.add,
                )
            else:
                o = tmps.tile([P, QC], F16, name="otile", tag="otile", bufs=1)
                nc.vector.tensor_scalar(
                    out=o,
                    in0=t1,
                    scalar1=bcol(gcol + mt),
                    scalar2=bcol(scol + mt),
                    op0=OP.mult,
                    op1=OP.add,
                )
                nc.sync.dma_start(
                    out=out_dram[mt * P : (mt + 1) * P, qc * QC : (qc + 1) * QC], in_=o
                )

    def out_proj(YT, wname, bc, resid):
        """resid[mo, q] += sum_hd Wp[hd, mo] YT[hd, q] + bp[mo] (in place)."""
        pre = resid
        w = [load_w(wname, t) for t in range(NT)]
        for ot in range(NT):
            pss = [psPr.tile([P, QC], F32, name="pr", tag="pr") for _ in range(NQC)]
            for ht in range(NT):
                for qc in range(NQC):
                    nc.tensor.matmul(
                        pss[qc],
                        lhsT=w[ht][:, ot * P : (ot + 1) * P],
                        rhs=YT[ht][:, qc * QC : (qc + 1) * QC],
                        start=(ht == 0),
                        stop=(ht == NT - 1),
                    )
            for qc in range(NQC):
                ps = pss[qc]
                t = tmps.tile([P, QC], F16, name="lnt", tag="lnt", bufs=3)
                nc.vector.tensor_scalar(
                    out=t, in0=ps, scalar1=bcol(bc + ot), scalar2=None, op0=OP.add
                )
                nc.gpsimd.tensor_tensor(
                    out=pre[ot][:, qc * QC : (qc + 1) * QC],
                    in0=t,
                    in1=resid[ot][:, qc * QC : (qc + 1) * QC],
                    op=OP.add,
                )

    # ================= self-attention =================
    QT = [acts.tile([P, SEQ], F16, name="QT", tag="QT", bufs=NT) for _ in range(NT)]
    KT = [acts.tile([P, SEQ], F16, name="KT", tag="KT", bufs=NT) for _ in range(NT)]
    vaug = [acts.tile([P, NH, DH + 1], F16, name="vaug", tag="vaug", bufs=NT) for _ in range(NT)]
    for kt in range(NT):
        nc.vector.memset(vaug[kt][:, :, DH : DH + 1], 1.0)
    with nc.named_scope("selfQKV"):
        proj_T(QT, xT, "wq1", _BQ1, w_pre=wq1_pre)
        proj_T(KT, xT, "wk1", _BK1)
        proj_V(vaug, xT, "wv1", 0)
    YT = [acts.tile([P, SEQ], F16, name="YT", tag="YT", bufs=NT) for _ in range(NT)]
    with nc.named_scope("selfAttn"):
        attention(QT, KT, vaug, YT, self_blocks, masked=True)
    with nc.named_scope("selfOut"):
        out_proj(YT, "wp1", _BP1, xT)  # xT becomes o1pre in place
    o1T = [acts.tile([P, SEQ], F16, name="o1T", tag="o1T", bufs=NT) for _ in range(NT)]
    with nc.named_scope("ln1"):
        for qc in range(NQC):
            layernorm(xT, o1T, _G1, _S1, qc)

    # ================= cross-attention =================
    eT = []
    eTd = io["eT"].rearrange("(t p) q -> t p q", p=P)
    for t in range(NT):
        tl = acts.tile([P, SEQ], F16, name="eT", tag="eT", bufs=NT)
        nc.sync.dma_start(out=tl, in_=eTd[t])
        eT.append(tl)
    QT2 = [acts.tile([P, SEQ], F16, name="QT", tag="QT", bufs=NT) for _ in range(NT)]
    KT2 = [acts.tile([P, SEQ], F16, name="KT", tag="KT", bufs=NT) for _ in range(NT)]
    vaug2 = [acts.tile([P, NH, DH + 1], F16, name="vaug", tag="vaug", bufs=NT) for _ in range(NT)]
    for kt in range(NT):
        nc.vector.memset(vaug2[kt][:, :, DH : DH + 1], 1.0)
    # K/V first: they depend only on enc, so the PE works on them while the
    # DVE/ACT tail of LN1 finishes; Q (which needs o1T) comes last.
    with nc.named_scope("crossKV"):
        proj_T(KT2, eT, "wk2", _BK2)
        proj_V(vaug2, eT, "wv2", 1)
    with nc.named_scope("crossQ"):
        proj_T(QT2, o1T, "wq2", _BQ2)
    all_blocks = [[(kt, False) for kt in range(NT)] for _ in range(NQC)]
    YT2 = [acts.tile([P, SEQ], F16, name="YT", tag="YT", bufs=NT) for _ in range(NT)]
    with nc.named_scope("crossAttn"):
        attention(QT2, KT2, vaug2, YT2, all_blocks, masked=False)
    with nc.named_scope("crossOut"):
        out_proj(YT2, "wp2", _BP2, o1T)  # o1T becomes o2pre in place
    o2T = [acts.tile([P, SEQ], F16, name="o2T", tag="xT", bufs=NT) for _ in range(NT)]
    with nc.named_scope("ln2"):
        for qc in range(NQC):
            layernorm(o1T, o2T, _G2, _S2, qc)

    # ================= FFN =================
    FH = FT // 2  # 16 f-tiles per half
    o3pre = o2T  # o3pre overwrites o2T in place (after all reads of each chunk)
    ffn_scope = nc.named_scope("ffn")
    ffn_scope.__enter__()
    for qc in range(NQC):
        o3h = []  # fp32 partial sums for the first f-half
        for fh in range(2):
            # produce hT for this (qc, fh): 16 tiles of [P, QC] fp16
            hts = []
            for fi in range(FH):
                ft = fh * FH + fi
                w1 = wstr.tile([P, NT, P], F16, name="w", tag="w", bufs=8)
                nc.sync.dma_start(out=w1, in_=io["w1r"][ft])
                ph = psS.tile([P, QC], F32, name="s", tag="s", bufs=2)
                for mt in range(NT):
                    nc.tensor.matmul(
                        ph,
                        lhsT=w1[:, mt, :],
                        rhs=o2T[mt][:, qc * QC : (qc + 1) * QC],
                        start=(mt == 0),
                        stop=(mt == NT - 1),
                    )
                if fi % 2 == 0:
                    hpair = acts.tile([P, 2, QC], F16, name="QT", tag="QT", bufs=NT)
                h = hpair[:, fi % 2, :]
                nc.scalar.activation(h, ph, AF.Relu, bias=bcol(_B1 + ft))
                hts.append(h)
            # consume: o3 accumulation over this f-half
            for mo in range(NT):
                ps = psPr.tile([P, QC], F32, name="pr", tag="pr")
                w2c = []
                for half in range(2):
                    f0 = fh * FH + half * 8
                    w2 = wstr.tile([P, 8, P], F16, name="w2c", tag="w", bufs=8)
                    nc.sync.dma_start(
                        out=w2,
                        in_=io["w2b"][mo, f0 : f0 + 8].rearrange("f p c -> p f c"),
                    )
                    w2c.append(w2)
                for fi in range(FH):
                    nc.tensor.matmul(
                        ps,
                        lhsT=w2c[fi // 8][:, fi % 8, :],
                        rhs=hts[fi],
                        start=(fi == 0),
                        stop=(fi == FH - 1),
                    )
                if fh == 0:
                    o = acts.tile([P, QC], F32, name="eT", tag="eT", bufs=NT)
                    nc.vector.tensor_scalar(
                        out=o,
                        in0=ps,
                        scalar1=bcol(_B2 + mo),
                        scalar2=None,
                        op0=OP.add,
                    )
                    o3h.append(o)
                else:
                    t = tmps.tile([P, QC], F16, name="lnt", tag="lnt", bufs=3)
                    nc.vector.tensor_tensor(out=t, in0=ps, in1=o3h[mo], op=OP.add)
                    nc.vector.tensor_tensor(
                        out=o3pre[mo][:, qc * QC : (qc + 1) * QC],
                        in0=t,
                        in1=o2T[mo][:, qc * QC : (qc + 1) * QC],
                        op=OP.add,
                    )
        layernorm(o3pre, None, _G3, _S3, qc, out_dram=io["o3T"])
    ffn_scope.__exit__(None, None, None)


def _analyze_mask(mask):
    """Per q-chunk, the contributing k-tiles for self-attention and whether
    each needs the additive mask.  Must be consistent across all cores
    (falls back to fully-mixed otherwise)."""
    blocks = []
    any_mixed = False
    for qc in range(NQC):
        lst = []
        for kt in range(NT):
            sub = mask[:, qc * QC : (qc + 1) * QC, kt * P : (kt + 1) * P]
            if sub.all():
                continue  # fully masked on every core -> contributes nothing
            mixed = bool(sub.any())
            any_mixed = any_mixed or mixed
            lst.append((kt, mixed))
        blocks.append(lst)
    return blocks, any_mixed


def _build(self_blocks):
    nc = bacc.Bacc(
        "TRN2",
        target_bir_lowering=False,
        debug=False,
        num_devices=NB,
    )
    io = {}
    io["xT"] = nc.dram_tensor("xT", [EMB, SEQ], F16, kind="ExternalInput").ap()
    io["eT"] = nc.dram_tensor("eT", [EMB, SEQ], F16, kind="ExternalInput").ap()
    io["mT"] = nc.dram_tensor("mT", [SEQ, SEQ], F16, kind="ExternalInput").ap()
    for w in ("wq1", "wk1", "wv1", "wp1", "wq2", "wk2", "wv2", "wp2"):
        io[w] = nc.dram_tensor(w, [EMB, EMB], F16, kind="ExternalInput").ap()
    io["w1r"] = nc.dram_tensor("w1r", [FT, P, NT, P], F16, kind="ExternalInput").ap()
    io["w2b"] = nc.dram_tensor("w2b", [NT, FT, P, P], F16, kind="ExternalInput").ap()
    io["bpack"] = nc.dram_tensor("bpack", [P, NBCOL], F32, kind="ExternalInput").ap()
    io["bvrow"] = nc.dram_tensor("bvrow", [2, EMB], F16, kind="ExternalInput").ap()
    io["o3T"] = nc.dram_tensor("o3T", [EMB, SEQ], F16, kind="ExternalOutput").ap()

    with tile.TileContext(nc) as tc:
        with ExitStack() as ctx:
            _emit(tc, ctx, io, self_blocks)
    nc.compile()
    return nc


def _prep_inputs(inputs):
    """Host-side prep: per-core in_maps with transposed/retiled fp16 arrays."""
    f16 = np.float16
    dec, enc, mask = inputs["dec_inp"], inputs["enc_inp"], inputs["mask"]
    mask = np.asarray(mask).astype(bool)
    self_blocks, _ = _analyze_mask(mask)

    def headcat(w):  # [H, M, DH] -> [M, H*DH]
        return np.ascontiguousarray(
            np.asarray(w).transpose(1, 0, 2).reshape(EMB, EMB)
        ).astype(f16)

    shared = {}
    for tag in ("1", "2"):
        shared["wq" + tag] = headcat(inputs["Wq" + tag])
        shared["wk" + tag] = headcat(inputs["Wk" + tag])
        shared["wv" + tag] = headcat(inputs["Wv" + tag])
        shared["wp" + tag] = np.ascontiguousarray(inputs["Wp" + tag]).astype(f16)
    W1, W2 = np.asarray(inputs["W1"]), np.asarray(inputs["W2"])
    shared["w1r"] = np.ascontiguousarray(
        W1.reshape(NT, P, FT, P).transpose(2, 1, 0, 3)
    ).astype(f16)
    shared["w2b"] = np.ascontiguousarray(
        W2.reshape(FT, P, NT, P).transpose(2, 0, 1, 3)
    ).astype(f16)

    def cols(v):  # [n*128] -> [128, n] per-partition layout
        v = np.asarray(v).astype(np.float32)
        return v.reshape(-1, P).T

    bpack = np.zeros((P, NBCOL), np.float32)
    bpack[:, _BQ1:_BK1] = cols(np.asarray(inputs["bq1"]).reshape(-1))
    bpack[:, _BK1:_BP1] = cols(np.asarray(inputs["bk1"]).reshape(-1))
    bpack[:, _BP1:_BQ2] = cols(inputs["bp1"])
    bpack[:, _BQ2:_BK2] = cols(np.asarray(inputs["bq2"]).reshape(-1))
    bpack[:, _BK2:_BP2] = cols(np.asarray(inputs["bk2"]).reshape(-1))
    bpack[:, _BP2:_B1] = cols(inputs["bp2"])
    bpack[:, _B1:_B2] = cols(inputs["b1"])
    bpack[:, _B2 : _B2 + 8] = cols(inputs["b2"])
    bpack[:, _G1:_S1] = cols(inputs["g1"])
    bpack[:, _S1:_G2] = cols(inputs["s1"])
    bpack[:, _G2:_S2] = cols(inputs["g2"])
    bpack[:, _S2:_G3] = cols(inputs["s2"])
    bpack[:, _G3:_S3] = cols(inputs["g3"])
    bpack[:, _S3:NBCOL] = cols(inputs["s3"])
    shared["bpack"] = bpack
    shared["bvrow"] = np.stack(
        [
            np.asarray(inputs["bv1"]).reshape(-1),
            np.asarray(inputs["bv2"]).reshape(-1),
        ]
    ).astype(np.float16)

    in_maps = []
    for n in range(NB):
        m = dict(shared)
        m["xT"] = np.ascontiguousarray(np.asarray(dec[n]).T).astype(f16)
        m["eT"] = np.ascontiguousarray(np.asarray(enc[n]).T).astype(f16)
        m["mT"] = np.where(mask[n].T, np.float16(0.0), np.float16(1.0))
        in_maps.append(m)
    return in_maps, self_blocks


def kernel(**inputs):
    in_maps, self_blocks = _prep_inputs(inputs)
    nc = _build(self_blocks)
    # the first execution after another process crashed the device can hit a
    # transient NRT_EXEC_UNIT_UNRECOVERABLE; a retry has always recovered it
    last = None
    for attempt in range(3):
        try:
            res = bass_utils.run_bass_kernel_spmd(
                nc, in_maps, core_ids=list(range(NB))
            )
            break
        except Exception as e:  # noqa: BLE001
            last = e
            time.sleep(3)
    else:
        raise last
    out = np.stack([np.asarray(r["o3T"]).T for r in res.results])
    return out.astype(np.float32)

